# revision 25
# baseline (speedup 1.0000x reference)
"""CantorAttention Trainium2 kernel (8 NeuronCores, SPMD).

Strategy
--------
Shard (batch=2) x (head-pairs=4) across the 8 cores: core c handles batch
c//4 and heads {2*(c%4), 2*(c%4)+1}.  QKV projection is column-sharded,
output projection row-sharded per head pair; partial outputs are summed on
host.

The sparse gather `k[:, :, routes, :]` is turned into *dense band attention*
by a host-side permutation: sorting positions so that each query's K=64
routed keys fall in a small contiguous window (for the Cantor-route
structure, a 128-aligned window of <=3 x 128 keys per 128-query tile).
Duplicate / arbitrary routes are handled exactly via a per-(query,key)
count mask multiplied into exp(scores); unstructured routes degrade
gracefully to the full dense 2048-key window.

Device dataflow per core (bf16 compute, f32 PSUM accumulate):
  xT (512,2048)  = x[b].T with permuted columns (host-prepped)
  qkvT = Wqkv_c.T @ xT      -> q^T,k^T,v^T with head_dim on partitions
  v    = per-128 transpose of v^T (+ ones columns for the softmax Z)
  key-chunk-major scores: for key chunk g, the covering queries form a
  contiguous range (<=512 wide):  S^T = k^T_g.T @ q^T_range   (one matmul)
    P^T = exp(S*scale) * count_mask                           (ACT + DVE)
  per query tile t (once its last chunk is done), per head:
    attn_unnorm | Z = P^T_chunks.T @ [V | 1]   (PE, accumulated)
    attn = attn_unnorm * (1/Z)                 (DVE)
  groups of 4 tiles: aT = attn^T (PE transpose), out^T = Wout-chunks @ aT
  DMA out^T (512, 2048) bf16; host un-permutes, sums partials, adds biases.
"""

import numpy as np
import ml_dtypes

import concourse.bass as bass
import concourse.tile as tile
from concourse import bacc, mybir, masks
from concourse.bass_utils import run_bass_kernel_spmd

BF16 = ml_dtypes.bfloat16
B, S, DIM, H, HD, KNN = 2, 2048, 512, 8, 64, 64
NCORES = 8
T = 128           # queries per tile
NT = S // T       # 16 query tiles
NG = S // T       # 16 key chunks
SCALE = 1.0 / float(np.sqrt(HD))
CCH = DIM // 128  # 4 contraction chunks of the model dim
VSTR = 132        # v block stride: [v_h0 64 | ones 2 | v_h1 64 | ones 2]


# ----------------------------------------------------------------------------
# Host-side planning: permutation + per-tile key windows + count masks
# ----------------------------------------------------------------------------

def _cantor_perm() -> np.ndarray:
    """Sort order of positions by their Cantor-set coordinate (the structure
    the reference's routes are built from)."""
    x = np.arange(S, dtype=np.float64) / max(1, S - 1)
    x = np.clip(x, 1e-06, 1.0 - 1e-06)
    val = np.zeros(S, dtype=np.float64)
    factor = 0.5
    for _ in range(8):
        x *= 3.0
        digit = np.floor(x)
        x -= digit
        val += (digit == 2.0) * factor
        factor *= 0.5
    return np.argsort(val.astype(np.float32), kind="stable")


def _windows_for(perm: np.ndarray, routes: np.ndarray):
    inv = np.empty(S, np.int64)
    inv[perm] = np.arange(S)
    r_q = inv[routes][perm]  # (S, K): sorted-query -> sorted key positions
    lo = np.empty(NT, np.int64)
    nkc = np.empty(NT, np.int64)
    for t in range(NT):
        blk = r_q[t * T:(t + 1) * T]
        lo[t] = (blk.min() // T) * T
        nkc[t] = -(-(blk.max() + 1 - lo[t]) // T)
    return r_q, lo, nkc


class Plan:
    pass


def _plan(routes: np.ndarray) -> Plan:
    candidates = [
        _cantor_perm(),
        np.arange(S),
        np.argsort(routes.min(axis=1), kind="stable"),
        np.argsort(np.median(routes, axis=1), kind="stable"),
    ]
    best = None
    for perm in candidates:
        r_q, lo, nkc = _windows_for(perm, routes)
        cost = int(nkc.sum())
        if best is None or cost < best[0]:
            best = (cost, perm, r_q, lo, nkc)
    _, perm, r_q, lo, nkc = best

    def covers_of(lo, nkc):
        cover = [[] for _ in range(NG)]
        for t in range(NT):
            for kc in range(int(nkc[t])):
                cover[int(lo[t]) // T + kc].append(t)
        return cover

    cover = covers_of(lo, nkc)
    if any(ts != list(range(ts[0], ts[0] + len(ts))) for ts in cover if ts):
        # adversarial routes: windows interleave; use full dense windows
        lo = np.zeros(NT, np.int64)
        nkc = np.full(NT, NG, np.int64)
        cover = covers_of(lo, nkc)

    p = Plan()
    p.perm, p.lo, p.nkc = perm, lo, nkc

    # score jobs: (g, t0, nt) pieces with nt <= 4 (N <= 512)
    pieces = []
    for g in range(NG):
        ts = cover[g]
        if not ts:
            continue
        i = 0
        while i < len(ts):
            nt = min(4, len(ts) - i)
            pieces.append((g, ts[i], nt))
            i += nt

    # structured case: g-major emission, all P^T tiles held in SBUF (phased).
    # dense case: (t0, g)-major emission with interleaved PV to bound liveness.
    p.phased = len(pieces) <= 24
    if not p.phased:
        pieces.sort(key=lambda x: (x[1], x[0]))

    jobs = []            # (g, t0, nt, block_base)
    piece_of = {}        # (g, t) -> (job_idx, t0)
    nblocks = 0
    for g, t0, nt in pieces:
        jidx = len(jobs)
        jobs.append((g, t0, nt, nblocks))
        for t in range(t0, t0 + nt):
            piece_of[(g, t)] = (jidx, t0)
        nblocks += nt
    p.jobs, p.piece_of, p.nblocks = jobs, piece_of, nblocks

    # PV emission point per tile: after its last-arriving piece
    p.emit_after_job = [[] for _ in range(len(jobs))]
    for t in range(NT):
        jmax = max(piece_of[(int(lo[t]) // T + kc, t)][0]
                   for kc in range(int(nkc[t])))
        p.emit_after_job[jmax].append(t)

    # count masks, in job-block order: mask[key_in_chunk, query_in_tile]
    maskG = np.zeros((nblocks, T, T), np.float32)
    for g, t0, nt, base in jobs:
        for j, t in enumerate(range(t0, t0 + nt)):
            blk = r_q[t * T:(t + 1) * T]
            sel = (blk // T) == g
            w = (blk % T)[sel]
            q_idx = np.broadcast_to(np.arange(T)[:, None], blk.shape)[sel]
            np.add.at(maskG, (base + j, w, q_idx), 1.0)
    p.maskG = maskG.astype(BF16)
    return p


# ----------------------------------------------------------------------------
# Device program
# ----------------------------------------------------------------------------

def _build(p: Plan, with_qk_bias: bool):
    f32 = mybir.dt.float32
    bf16 = mybir.dt.bfloat16
    lo, nkc = p.lo, p.nkc
    nc = bacc.Bacc("TRN2", target_bir_lowering=False, debug=False,
                   num_devices=NCORES)

    xT_d = nc.dram_tensor("xT", [DIM, S], bf16, kind="ExternalInput").ap()
    wqkv_d = nc.dram_tensor("wqkv", [DIM, 384], bf16, kind="ExternalInput").ap()
    wout_d = nc.dram_tensor("wout", [128, DIM], bf16, kind="ExternalInput").ap()
    maskG_d = nc.dram_tensor("maskG", [p.nblocks, T, T], bf16,
                             kind="ExternalInput").ap()
    if with_qk_bias:
        bqk_d = nc.dram_tensor("bqk", [256, 1], f32, kind="ExternalInput").ap()
    outT_d = nc.dram_tensor("outT", [DIM, S], bf16, kind="ExternalOutput").ap()

    ptg_bufs = len(p.jobs) if p.phased else 20

    with tile.TileContext(nc) as tc:
        with (
            tc.tile_pool(name="persist", bufs=1) as persist,
            tc.tile_pool(name="ps_s2", bufs=3, space="PSUM") as ps_s2,
            tc.tile_pool(name="ps_o", bufs=2, space="PSUM") as ps_o,
            tc.tile_pool(name="mask", bufs=16) as maskp,
            tc.tile_pool(name="ptg", bufs=ptg_bufs) as ptgp,
            tc.tile_pool(name="small", bufs=16) as smallp,
        ):
            xT = persist.tile([128, CCH * S], bf16, tag="xT")
            qkT = persist.tile([128, 2 * S], bf16, tag="qkT")
            vT = persist.tile([128, S], bf16, tag="vT")
            v_sb = persist.tile([128, NT * VSTR], bf16, tag="v")
            wqkv = persist.tile([128, CCH * 384], bf16, tag="wqkv")
            wout = persist.tile([128, DIM], bf16, tag="wout")
            outT = persist.tile([128, CCH * S], bf16, tag="outT")
            ident = persist.tile([128, 128], bf16, tag="ident")

            masks.make_identity(nc, ident[:])

            nc.sync.dma_start(
                wqkv[:].rearrange("p (c f) -> p c f", c=CCH),
                wqkv_d.rearrange("(c p) f -> p c f", p=128))
            for c in range(CCH):
                nc.sync.dma_start(xT[:, c * S:(c + 1) * S],
                                  xT_d[c * 128:(c + 1) * 128, :])
            nc.sync.dma_start(wout[:], wout_d)
            if with_qk_bias:
                bqk = persist.tile([128, 2], f32, tag="bqk")
                nc.sync.dma_start(
                    bqk[:].rearrange("p (c f) -> p c f", c=2),
                    bqk_d.rearrange("(c p) f -> p c f", p=128))

            # ---- stage A: qkvT = Wqkv_c.T @ xT  (3 f-tiles: q|k|v pairs) ----
            for f in (0, 1, 2):
                for np2 in range(2):
                    ps = ps_s2.tile([128, 1024], f32, tag="S2")
                    for c in range(CCH):
                        for j2 in range(2):
                            n = np2 * 2 + j2
                            nc.tensor.matmul(
                                ps[:, j2 * 512:(j2 + 1) * 512],
                                lhsT=wqkv[:, c * 384 + f * 128:
                                          c * 384 + (f + 1) * 128],
                                rhs=xT[:, c * S + n * 512:c * S + (n + 1) * 512],
                                start=(c == 0), stop=(c == CCH - 1))
                    if f < 2:
                        dst = qkT[:, f * S + np2 * 1024:f * S + (np2 + 1) * 1024]
                    else:
                        dst = vT[:, np2 * 1024:(np2 + 1) * 1024]
                    if with_qk_bias and f < 2:
                        nc.vector.tensor_scalar_add(dst, ps[:], bqk[:, f:f + 1])
                    elif np2 == 0:
                        nc.scalar.copy(dst, ps[:])
                    else:
                        nc.vector.tensor_copy(dst, ps[:])

            # ---- stage B: v natural blocks via PE transpose of vT ----
            nc.vector.memset(
                v_sb[:].rearrange("p (g f) -> p g f", g=2 * NT)[:, :, 64:66],
                1.0)
            for g in range(NT):
                psv = ps_o.tile([128, 128], bf16, tag="O")
                nc.tensor.transpose(psv[:], vT[:, g * 128:(g + 1) * 128], ident[:])
                nc.vector.tensor_copy(
                    v_sb[:, g * VSTR:g * VSTR + VSTR].rearrange(
                        "p (h f) -> p h f", h=2)[:, :, 0:64],
                    psv[:].rearrange("p (h f) -> p h f", h=2))

            # ---- stage C: scores + exp + mask; PV; transpose + projection ----
            pt_tiles = {}
            attn_tiles = {}

            def emit_job(jidx):
                g, t0, nt, base = p.jobs[jidx]
                nq = nt * 128
                mt = maskp.tile([128, 512], bf16, tag="mask")
                nc.sync.dma_start(
                    mt[:, 0:nq].rearrange("p (a f) -> p a f", a=nt),
                    maskG_d[base:base + nt].rearrange("a p f -> p a f"))
                pss = ps_s2.tile([128, 1024], f32, tag="S2")
                for h in range(2):
                    hp = h * 64
                    nc.tensor.matmul(
                        pss[:, h * 512:h * 512 + nq],
                        lhsT=qkT[hp:hp + 64, S + g * 128:S + (g + 1) * 128],
                        rhs=qkT[hp:hp + 64, t0 * 128:t0 * 128 + nq],
                        start=True, stop=True)
                pt = ptgp.tile([128, 1024], bf16, tag="ptg")
                pt3 = pt[:, 0:2 * nq].rearrange("p (h f) -> p h f", h=2)
                nc.scalar.activation(
                    pt3, pss[:].rearrange("p (h f) -> p h f", h=2)[:, :, 0:nq],
                    mybir.ActivationFunctionType.Exp, scale=SCALE)
                nc.vector.tensor_mul(pt[:, 0:nq], pt[:, 0:nq], mt[:, 0:nq])
                nc.vector.tensor_mul(pt[:, nq:2 * nq], pt[:, nq:2 * nq],
                                     mt[:, 0:nq])
                pt_tiles[jidx] = (pt, nq)

            def emit_pv(t):
                attn = smallp.tile([128, 128], bf16, tag="attn")
                attn_tiles[t] = attn
                nk = int(nkc[t])
                for h in range(2):
                    pso = ps_o.tile([128, 68], f32, tag="O")
                    for kc in range(nk):
                        gg = int(lo[t]) // T + kc
                        jj, tt0 = p.piece_of[(gg, t)]
                        ptile, jnq = pt_tiles[jj]
                        coff = h * jnq + (t - tt0) * 128
                        nc.tensor.matmul(
                            pso[:, 0:65],
                            lhsT=ptile[:, coff:coff + 128],
                            rhs=v_sb[:, gg * VSTR + h * 66:
                                     gg * VSTR + h * 66 + 65],
                            start=(kc == 0), stop=(kc == nk - 1))
                    rz = smallp.tile([128, 1], f32, tag="rz")
                    nc.vector.reciprocal(rz[:], pso[:, 64:65])
                    if h == 0:
                        nc.vector.tensor_scalar_mul(
                            attn[:, 0:64], pso[:, 0:64], rz[:])
                    else:
                        nc.scalar.mul(attn[:, 64:128], pso[:, 0:64], rz[:])

            def emit_group(tgrp):
                aT_wide = smallp.tile([128, 512], bf16, tag="aTw")
                for j in range(4):
                    t = tgrp * 4 + j
                    pst = ps_o.tile([128, 128], bf16, tag="O")
                    nc.tensor.transpose(pst[:], attn_tiles[t][:], ident[:])
                    if j % 2 == 0:
                        nc.scalar.copy(aT_wide[:, j * 128:(j + 1) * 128], pst[:])
                    else:
                        nc.vector.tensor_copy(
                            aT_wide[:, j * 128:(j + 1) * 128], pst[:])
                for op2 in range(CCH // 2):
                    psp = ps_s2.tile([128, 1024], f32, tag="S2")
                    for j2 in range(2):
                        oc = op2 * 2 + j2
                        nc.tensor.matmul(
                            psp[:, j2 * 512:(j2 + 1) * 512],
                            lhsT=wout[:, oc * 128:(oc + 1) * 128],
                            rhs=aT_wide[:], start=True, stop=True)
                    dst3 = outT[:].rearrange(
                        "p (c f) -> p c f", c=CCH)[
                        :, op2 * 2:op2 * 2 + 2, tgrp * 512:(tgrp + 1) * 512]
                    src3 = psp[:].rearrange("p (h f) -> p h f", h=2)
                    if op2 == 0:
                        nc.vector.tensor_copy(dst3, src3)
                    else:
                        nc.scalar.copy(dst3, src3)
                nc.sync.dma_start(
                    outT_d.rearrange("(c p) f -> p c f", p=128)
                          [:, :, tgrp * 512:(tgrp + 1) * 512],
                    outT[:].rearrange("p (c f) -> p c f", c=CCH)
                        [:, :, tgrp * 512:(tgrp + 1) * 512])

            if p.phased:
                for jidx in range(len(p.jobs)):
                    emit_job(jidx)
                for t in range(NT):
                    emit_pv(t)
                    if t % 4 == 3:
                        emit_group(t // 4)
            else:
                done = [0] * (NT // 4)
                for jidx in range(len(p.jobs)):
                    emit_job(jidx)
                    for t in p.emit_after_job[jidx]:
                        emit_pv(t)
                        done[t // 4] += 1
                        if done[t // 4] == 4:
                            emit_group(t // 4)

    nc.compile()
    return nc


_CACHE = {}


def _get_program(p: Plan, with_qk_bias: bool):
    key = (tuple(int(v) for v in p.lo), tuple(int(v) for v in p.nkc),
           bool(with_qk_bias))
    if key not in _CACHE:
        _CACHE[key] = _build(p, with_qk_bias)
    return _CACHE[key]


# ----------------------------------------------------------------------------
# Entry point
# ----------------------------------------------------------------------------

def kernel(x, Wqkv, bqkv, Wout, bout, routes):
    x = np.asarray(x, np.float32)
    Wqkv = np.asarray(Wqkv, np.float32)
    bqkv = np.asarray(bqkv, np.float32)
    Wout = np.asarray(Wout, np.float32)
    bout = np.asarray(bout, np.float32)
    routes = np.asarray(routes)

    p = _plan(routes)
    perm = p.perm

    bq = bqkv[0:DIM]
    bk = bqkv[DIM:2 * DIM]
    bv = bqkv[2 * DIM:3 * DIM]
    with_qk_bias = bool(np.any(bq) or np.any(bk))

    nc = _get_program(p, with_qk_bias)

    maskG_flat = np.ascontiguousarray(p.maskG)
    in_maps = []
    for c in range(NCORES):
        b = c // 4
        h0 = 2 * (c % 4)
        cols = slice(h0 * HD, (h0 + 2) * HD)
        wqkv = np.concatenate(
            [Wqkv[:, cols],
             Wqkv[:, DIM + h0 * HD:DIM + (h0 + 2) * HD],
             Wqkv[:, 2 * DIM + h0 * HD:2 * DIM + (h0 + 2) * HD]], axis=1)
        m = {
            "xT": np.ascontiguousarray(x[b].T[:, perm]).astype(BF16),
            "wqkv": np.ascontiguousarray(wqkv).astype(BF16),
            "wout": np.ascontiguousarray(
                Wout[h0 * HD:(h0 + 2) * HD, :]).astype(BF16),
            "maskG": maskG_flat,
        }
        if with_qk_bias:
            m["bqk"] = np.concatenate(
                [bq[h0 * HD:(h0 + 2) * HD],
                 bk[h0 * HD:(h0 + 2) * HD]]).reshape(256, 1).astype(np.float32)
        in_maps.append(m)

    global _last_in_maps
    _last_in_maps = in_maps
    res = run_bass_kernel_spmd(nc, in_maps, core_ids=list(range(NCORES)))

    out = np.zeros((B, S, DIM), np.float32)
    for c in range(NCORES):
        b = c // 4
        part = res.results[c]["outT"].astype(np.float32).T  # (S, DIM) permuted
        out[b][perm] += part
    out += bout[None, None, :]
    if np.any(bv):
        out += (bv @ Wout)[None, None, :]
    return out


# revision 26
# speedup vs baseline: 1.0291x; 1.0291x over previous
"""CantorAttention Trainium2 kernel (8 NeuronCores, SPMD).

Strategy
--------
Shard (batch=2) x (head-pairs=4) across the 8 cores: core c handles batch
c//4 and heads {2*(c%4), 2*(c%4)+1}.  QKV projection is column-sharded,
output projection row-sharded per head pair; partial outputs are summed on
host.

The sparse gather `k[:, :, routes, :]` is turned into *dense band attention*
by a host-side permutation: sorting positions so that each query's K=64
routed keys fall in a small contiguous window (for the Cantor-route
structure, a 128-aligned window of <=3 x 128 keys per 128-query tile).
Duplicate / arbitrary routes are handled exactly via a per-(query,key)
count mask multiplied into exp(scores); unstructured routes degrade
gracefully to the full dense 2048-key window.

Device dataflow per core (bf16 compute, f32 PSUM accumulate):
  xT (512,2048)  = x[b].T with permuted columns (host-prepped)
  qkvT = Wqkv_c.T @ xT      -> q^T,k^T,v^T with head_dim on partitions
  v    = per-128 transpose of v^T (+ ones columns for the softmax Z)
  key-chunk-major scores: for key chunk g, the covering queries form a
  contiguous range (<=512 wide):  S^T = k^T_g.T @ q^T_range   (one matmul)
    P^T = exp(S*scale) * count_mask                           (ACT + DVE)
  per query tile t (once its last chunk is done), per head:
    attn_unnorm | Z = P^T_chunks.T @ [V | 1]   (PE, accumulated)
    attn = attn_unnorm * (1/Z)                 (DVE)
  groups of 4 tiles: aT = attn^T (PE transpose), out^T = Wout-chunks @ aT
  DMA out^T (512, 2048) bf16; host un-permutes, sums partials, adds biases.
"""

import numpy as np
import ml_dtypes

import concourse.bass as bass
import concourse.tile as tile
from concourse import bacc, mybir, masks
from concourse.bass_utils import run_bass_kernel_spmd

BF16 = ml_dtypes.bfloat16
B, S, DIM, H, HD, KNN = 2, 2048, 512, 8, 64, 64
NCORES = 8
T = 128           # queries per tile
NT = S // T       # 16 query tiles
NG = S // T       # 16 key chunks
SCALE = 1.0 / float(np.sqrt(HD))
CCH = DIM // 128  # 4 contraction chunks of the model dim
VSTR = 132        # v block stride: [v_h0 64 | ones 2 | v_h1 64 | ones 2]


# ----------------------------------------------------------------------------
# Host-side planning: permutation + per-tile key windows + count masks
# ----------------------------------------------------------------------------

def _cantor_perm() -> np.ndarray:
    """Sort order of positions by their Cantor-set coordinate (the structure
    the reference's routes are built from)."""
    x = np.arange(S, dtype=np.float64) / max(1, S - 1)
    x = np.clip(x, 1e-06, 1.0 - 1e-06)
    val = np.zeros(S, dtype=np.float64)
    factor = 0.5
    for _ in range(8):
        x *= 3.0
        digit = np.floor(x)
        x -= digit
        val += (digit == 2.0) * factor
        factor *= 0.5
    return np.argsort(val.astype(np.float32), kind="stable")


def _windows_for(perm: np.ndarray, routes: np.ndarray):
    inv = np.empty(S, np.int64)
    inv[perm] = np.arange(S)
    r_q = inv[routes][perm]  # (S, K): sorted-query -> sorted key positions
    lo = np.empty(NT, np.int64)
    nkc = np.empty(NT, np.int64)
    for t in range(NT):
        blk = r_q[t * T:(t + 1) * T]
        lo[t] = (blk.min() // T) * T
        nkc[t] = -(-(blk.max() + 1 - lo[t]) // T)
    return r_q, lo, nkc


class Plan:
    pass


def _plan(routes: np.ndarray) -> Plan:
    candidates = [
        _cantor_perm(),
        np.arange(S),
        np.argsort(routes.min(axis=1), kind="stable"),
        np.argsort(np.median(routes, axis=1), kind="stable"),
    ]
    best = None
    for perm in candidates:
        r_q, lo, nkc = _windows_for(perm, routes)
        cost = int(nkc.sum())
        if best is None or cost < best[0]:
            best = (cost, perm, r_q, lo, nkc)
    _, perm, r_q, lo, nkc = best

    def covers_of(lo, nkc):
        cover = [[] for _ in range(NG)]
        for t in range(NT):
            for kc in range(int(nkc[t])):
                cover[int(lo[t]) // T + kc].append(t)
        return cover

    cover = covers_of(lo, nkc)
    if any(ts != list(range(ts[0], ts[0] + len(ts))) for ts in cover if ts):
        # adversarial routes: windows interleave; use full dense windows
        lo = np.zeros(NT, np.int64)
        nkc = np.full(NT, NG, np.int64)
        cover = covers_of(lo, nkc)

    p = Plan()
    p.perm, p.lo, p.nkc = perm, lo, nkc

    # score jobs: (g, t0, nt) pieces with nt <= 4 (N <= 512)
    pieces = []
    for g in range(NG):
        ts = cover[g]
        if not ts:
            continue
        i = 0
        while i < len(ts):
            nt = min(4, len(ts) - i)
            pieces.append((g, ts[i], nt))
            i += nt

    # structured case: g-major emission, all P^T tiles held in SBUF (phased).
    # dense case: (t0, g)-major emission with interleaved PV to bound liveness.
    p.phased = len(pieces) <= 24
    if not p.phased:
        pieces.sort(key=lambda x: (x[1], x[0]))

    jobs = []            # (g, t0, nt, block_base)
    piece_of = {}        # (g, t) -> (job_idx, t0)
    nblocks = 0
    for g, t0, nt in pieces:
        jidx = len(jobs)
        jobs.append((g, t0, nt, nblocks))
        for t in range(t0, t0 + nt):
            piece_of[(g, t)] = (jidx, t0)
        nblocks += nt
    p.jobs, p.piece_of, p.nblocks = jobs, piece_of, nblocks

    # PV emission point per tile: after its last-arriving piece
    p.emit_after_job = [[] for _ in range(len(jobs))]
    for t in range(NT):
        jmax = max(piece_of[(int(lo[t]) // T + kc, t)][0]
                   for kc in range(int(nkc[t])))
        p.emit_after_job[jmax].append(t)

    # count masks, in job-block order: mask[key_in_chunk, query_in_tile]
    maskG = np.zeros((nblocks, T, T), np.float32)
    for g, t0, nt, base in jobs:
        for j, t in enumerate(range(t0, t0 + nt)):
            blk = r_q[t * T:(t + 1) * T]
            sel = (blk // T) == g
            w = (blk % T)[sel]
            q_idx = np.broadcast_to(np.arange(T)[:, None], blk.shape)[sel]
            np.add.at(maskG, (base + j, w, q_idx), 1.0)
    p.maskG = maskG.astype(BF16)
    return p


# ----------------------------------------------------------------------------
# Device program
# ----------------------------------------------------------------------------

def _build(p: Plan, with_qk_bias: bool):
    f32 = mybir.dt.float32
    bf16 = mybir.dt.bfloat16
    lo, nkc = p.lo, p.nkc
    nc = bacc.Bacc("TRN2", target_bir_lowering=False, debug=False,
                   num_devices=NCORES)

    xT_d = nc.dram_tensor("xT", [DIM, S], bf16, kind="ExternalInput").ap()
    wqkv_d = nc.dram_tensor("wqkv", [DIM, 384], bf16, kind="ExternalInput").ap()
    wout_d = nc.dram_tensor("wout", [128, DIM], bf16, kind="ExternalInput").ap()
    maskG_d = nc.dram_tensor("maskG", [p.nblocks, T, T], bf16,
                             kind="ExternalInput").ap()
    if with_qk_bias:
        bqk_d = nc.dram_tensor("bqk", [256, 1], f32, kind="ExternalInput").ap()
    outT_d = nc.dram_tensor("outT", [DIM, S], bf16, kind="ExternalOutput").ap()

    ptg_bufs = len(p.jobs) if p.phased else 20

    with tile.TileContext(nc) as tc:
        with (
            tc.tile_pool(name="persist", bufs=1) as persist,
            tc.tile_pool(name="ps_s2", bufs=3, space="PSUM") as ps_s2,
            tc.tile_pool(name="ps_o", bufs=2, space="PSUM") as ps_o,
            tc.tile_pool(name="mask", bufs=16) as maskp,
            tc.tile_pool(name="ptg", bufs=ptg_bufs) as ptgp,
            tc.tile_pool(name="small", bufs=16) as smallp,
        ):
            xT = persist.tile([128, CCH * S], bf16, tag="xT")
            qkT = persist.tile([128, 2 * S], bf16, tag="qkT")
            vT = persist.tile([128, S], bf16, tag="vT")
            v_sb = persist.tile([128, NT * VSTR], bf16, tag="v")
            wqkv = persist.tile([128, CCH * 384], bf16, tag="wqkv")
            wout = persist.tile([128, DIM], bf16, tag="wout")
            outT = persist.tile([128, CCH * S], bf16, tag="outT")
            ident = persist.tile([128, 128], bf16, tag="ident")

            masks.make_identity(nc, ident[:])

            nc.sync.dma_start(
                wqkv[:].rearrange("p (c f) -> p c f", c=CCH),
                wqkv_d.rearrange("(c p) f -> p c f", p=128))
            for c in range(CCH):
                nc.sync.dma_start(xT[:, c * S:(c + 1) * S],
                                  xT_d[c * 128:(c + 1) * 128, :])
            nc.sync.dma_start(wout[:], wout_d)
            if with_qk_bias:
                bqk = persist.tile([128, 2], f32, tag="bqk")
                nc.sync.dma_start(
                    bqk[:].rearrange("p (c f) -> p c f", c=2),
                    bqk_d.rearrange("(c p) f -> p c f", p=128))

            # ---- stage A: qkvT = Wqkv_c.T @ xT  (3 f-tiles: q|k|v pairs) ----
            for f in (0, 1, 2):
                for np2 in range(2):
                    ps = ps_s2.tile([128, 1024], f32, tag="S2")
                    for c in range(CCH):
                        for j2 in range(2):
                            n = np2 * 2 + j2
                            nc.tensor.matmul(
                                ps[:, j2 * 512:(j2 + 1) * 512],
                                lhsT=wqkv[:, c * 384 + f * 128:
                                          c * 384 + (f + 1) * 128],
                                rhs=xT[:, c * S + n * 512:c * S + (n + 1) * 512],
                                start=(c == 0), stop=(c == CCH - 1))
                    if f < 2:
                        dst = qkT[:, f * S + np2 * 1024:f * S + (np2 + 1) * 1024]
                    else:
                        dst = vT[:, np2 * 1024:(np2 + 1) * 1024]
                    if with_qk_bias and f < 2:
                        nc.vector.tensor_scalar_add(dst, ps[:], bqk[:, f:f + 1])
                    elif np2 == 0:
                        nc.scalar.copy(dst, ps[:])
                    else:
                        nc.vector.tensor_copy(dst, ps[:])

            # ---- stage B: v natural blocks via PE transpose of vT ----
            nc.vector.memset(
                v_sb[:].rearrange("p (g f) -> p g f", g=2 * NT)[:, :, 64:66],
                1.0)
            for g in range(NT):
                psv = ps_o.tile([128, 128], bf16, tag="O")
                nc.tensor.transpose(psv[:], vT[:, g * 128:(g + 1) * 128], ident[:])
                nc.vector.tensor_copy(
                    v_sb[:, g * VSTR:g * VSTR + VSTR].rearrange(
                        "p (h f) -> p h f", h=2)[:, :, 0:64],
                    psv[:].rearrange("p (h f) -> p h f", h=2))

            # ---- stage C: scores + exp + mask; PV; transpose + projection ----
            pt_tiles = {}
            attn_tiles = {}

            def emit_job(jidx):
                g, t0, nt, base = p.jobs[jidx]
                nq = nt * 128
                mt = maskp.tile([128, 512], bf16, tag="mask")
                nc.sync.dma_start(
                    mt[:, 0:nq].rearrange("p (a f) -> p a f", a=nt),
                    maskG_d[base:base + nt].rearrange("a p f -> p a f"))
                pss = ps_s2.tile([128, 1024], f32, tag="S2")
                for h in range(2):
                    hp = h * 64
                    nc.tensor.matmul(
                        pss[:, h * 512:h * 512 + nq],
                        lhsT=qkT[hp:hp + 64, S + g * 128:S + (g + 1) * 128],
                        rhs=qkT[hp:hp + 64, t0 * 128:t0 * 128 + nq],
                        start=True, stop=True)
                pt = ptgp.tile([128, 1024], bf16, tag="ptg")
                pt3 = pt[:, 0:2 * nq].rearrange("p (h f) -> p h f", h=2)
                nc.scalar.activation(
                    pt3, pss[:].rearrange("p (h f) -> p h f", h=2)[:, :, 0:nq],
                    mybir.ActivationFunctionType.Exp, scale=SCALE)
                nc.vector.tensor_mul(pt[:, 0:nq], pt[:, 0:nq], mt[:, 0:nq])
                nc.vector.tensor_mul(pt[:, nq:2 * nq], pt[:, nq:2 * nq],
                                     mt[:, 0:nq])
                pt_tiles[jidx] = (pt, nq)

            def emit_pv(t):
                attn = smallp.tile([128, 128], bf16, tag="attn")
                attn_tiles[t] = attn
                nk = int(nkc[t])
                for h in range(2):
                    pso = ps_o.tile([128, 68], f32, tag="O")
                    for kc in range(nk):
                        gg = int(lo[t]) // T + kc
                        jj, tt0 = p.piece_of[(gg, t)]
                        ptile, jnq = pt_tiles[jj]
                        coff = h * jnq + (t - tt0) * 128
                        nc.tensor.matmul(
                            pso[:, 0:65],
                            lhsT=ptile[:, coff:coff + 128],
                            rhs=v_sb[:, gg * VSTR + h * 66:
                                     gg * VSTR + h * 66 + 65],
                            start=(kc == 0), stop=(kc == nk - 1))
                    rz = smallp.tile([128, 1], f32, tag="rz")
                    nc.vector.reciprocal(rz[:], pso[:, 64:65])
                    if h == 0:
                        nc.vector.tensor_scalar_mul(
                            attn[:, 0:64], pso[:, 0:64], rz[:])
                    else:
                        nc.scalar.mul(attn[:, 64:128], pso[:, 0:64], rz[:])

            def emit_group(tgrp):
                aT_wide = smallp.tile([128, 512], bf16, tag="aTw")
                for j in range(4):
                    t = tgrp * 4 + j
                    pst = ps_o.tile([128, 128], bf16, tag="O")
                    nc.tensor.transpose(pst[:], attn_tiles[t][:], ident[:])
                    if j % 2 == 0:
                        nc.scalar.copy(aT_wide[:, j * 128:(j + 1) * 128], pst[:])
                    else:
                        nc.vector.tensor_copy(
                            aT_wide[:, j * 128:(j + 1) * 128], pst[:])
                for op2 in range(CCH // 2):
                    psp = ps_s2.tile([128, 1024], f32, tag="S2")
                    for j2 in range(2):
                        oc = op2 * 2 + j2
                        nc.tensor.matmul(
                            psp[:, j2 * 512:(j2 + 1) * 512],
                            lhsT=wout[:, oc * 128:(oc + 1) * 128],
                            rhs=aT_wide[:], start=True, stop=True)
                    dst3 = outT[:].rearrange(
                        "p (c f) -> p c f", c=CCH)[
                        :, op2 * 2:op2 * 2 + 2, tgrp * 512:(tgrp + 1) * 512]
                    src3 = psp[:].rearrange("p (h f) -> p h f", h=2)
                    if op2 == 0:
                        nc.vector.tensor_copy(dst3, src3)
                    else:
                        nc.scalar.copy(dst3, src3)
                nc.sync.dma_start(
                    outT_d.rearrange("(c p) f -> p c f", p=128)
                          [:, :, tgrp * 512:(tgrp + 1) * 512],
                    outT[:].rearrange("p (c f) -> p c f", c=CCH)
                        [:, :, tgrp * 512:(tgrp + 1) * 512])

            if p.phased:
                for jidx in range(len(p.jobs)):
                    emit_job(jidx)
                for t in range(NT):
                    emit_pv(t)
                for tgrp in range(NT // 4):
                    emit_group(tgrp)
            else:
                done = [0] * (NT // 4)
                for jidx in range(len(p.jobs)):
                    emit_job(jidx)
                    for t in p.emit_after_job[jidx]:
                        emit_pv(t)
                        done[t // 4] += 1
                        if done[t // 4] == 4:
                            emit_group(t // 4)

    nc.compile()
    return nc


_CACHE = {}


def _get_program(p: Plan, with_qk_bias: bool):
    key = (tuple(int(v) for v in p.lo), tuple(int(v) for v in p.nkc),
           bool(with_qk_bias))
    if key not in _CACHE:
        _CACHE[key] = _build(p, with_qk_bias)
    return _CACHE[key]


# ----------------------------------------------------------------------------
# Entry point
# ----------------------------------------------------------------------------

def kernel(x, Wqkv, bqkv, Wout, bout, routes):
    x = np.asarray(x, np.float32)
    Wqkv = np.asarray(Wqkv, np.float32)
    bqkv = np.asarray(bqkv, np.float32)
    Wout = np.asarray(Wout, np.float32)
    bout = np.asarray(bout, np.float32)
    routes = np.asarray(routes)

    p = _plan(routes)
    perm = p.perm

    bq = bqkv[0:DIM]
    bk = bqkv[DIM:2 * DIM]
    bv = bqkv[2 * DIM:3 * DIM]
    with_qk_bias = bool(np.any(bq) or np.any(bk))

    nc = _get_program(p, with_qk_bias)

    maskG_flat = np.ascontiguousarray(p.maskG)
    in_maps = []
    for c in range(NCORES):
        b = c // 4
        h0 = 2 * (c % 4)
        cols = slice(h0 * HD, (h0 + 2) * HD)
        wqkv = np.concatenate(
            [Wqkv[:, cols],
             Wqkv[:, DIM + h0 * HD:DIM + (h0 + 2) * HD],
             Wqkv[:, 2 * DIM + h0 * HD:2 * DIM + (h0 + 2) * HD]], axis=1)
        m = {
            "xT": np.ascontiguousarray(x[b].T[:, perm]).astype(BF16),
            "wqkv": np.ascontiguousarray(wqkv).astype(BF16),
            "wout": np.ascontiguousarray(
                Wout[h0 * HD:(h0 + 2) * HD, :]).astype(BF16),
            "maskG": maskG_flat,
        }
        if with_qk_bias:
            m["bqk"] = np.concatenate(
                [bq[h0 * HD:(h0 + 2) * HD],
                 bk[h0 * HD:(h0 + 2) * HD]]).reshape(256, 1).astype(np.float32)
        in_maps.append(m)

    global _last_in_maps
    _last_in_maps = in_maps
    res = run_bass_kernel_spmd(nc, in_maps, core_ids=list(range(NCORES)))

    out = np.zeros((B, S, DIM), np.float32)
    for c in range(NCORES):
        b = c // 4
        part = res.results[c]["outT"].astype(np.float32).T  # (S, DIM) permuted
        out[b][perm] += part
    out += bout[None, None, :]
    if np.any(bv):
        out += (bv @ Wout)[None, None, :]
    return out
